# revision 1
# baseline (speedup 1.0000x reference)
"""Trainium2 Bass kernel for the Centroid (segment_reduce) problem.

new_centroid = 0.3 * (segment_sum(embed, y) / counts) + 0.7 * centroid
  embed [32768, 1024] f32, y [32768] int64 (0..999), centroid [1000, 1024] f32

Strategy (8 NeuronCores, data-parallel over batch):
  - core i gets embed rows [4096*i, 4096*(i+1)) (pre-laid-out as fp8 e4m3;
    exact-match encodings for |x| <= 240) and the matching y shard as f32.
  - scatter-add as a dense one-hot matmul on TensorE in fp8 DoubleRow
    mode (two 128-row K-subtiles per instruction):
        sums[c, d] = sum_b onehot[b, c] * embed[b, d]
    classes padded to 1024 (8 M-tiles of 128); a constant ones column is
    prepended to the embed tile so column 0 of the first pass's matmul
    output is the per-class count (counts come free with the sums; the
    one-hot and ones are exact in fp8, accumulation is f32 PSUM).
  - the local sums+counts live in 3 column chunks (432/432/176). After a
    chunk's 8 class tiles finish it is cast to bf16 and ReduceScattered
    across the 8 cores while the next chunk's matmuls run. Counts <= 256
    stay exact in bf16; bf16/fp8 rounding is well inside the 2e-2 budget.
  - per chunk, once its RS lands: mean = sums * (0.3 / count), then
    out = mean + 0.7 * centroid for the core's 128 owned rows.
  - host concatenates the 8 [128, 1024] shards and trims to 1000 rows.
"""

import numpy as np

import concourse.bacc as bacc
import concourse.mybir as mybir
import concourse.tile as tile
from concourse.bass_utils import run_bass_kernel_spmd

N_CORES = 8
C = 1000  # real classes
C_PAD = 1024  # padded classes (8 tiles of 128)
D = 1024  # embed dim
B = 32768  # total batch
B_LOC = B // N_CORES  # 4096 rows per core
P = 128
KT = B_LOC // P  # 32 k-tiles per core
KP = KT // 2  # 16 k-pairs; DoubleRow consumes [128, 2, cols] per matmul
MT = C_PAD // P  # 8 class tiles
CM = C_PAD // N_CORES  # 128 classes owned per core after ReduceScatter
FACTOR = 0.3
W = 1 + D + 15  # count col + sums + pad -> 1040 cols
# PSUM passes (bank-limited to 512 f32 columns each)
CHUNKS = [(0, 432), (432, 432), (864, 176)]  # all widths mult of 16
# collective chunking is decoupled from the PSUM passes: pass 0 goes out in
# its own ReduceScatter (hidden under the remaining matmuls); passes 1+2
# share one [1024, 608] buffer so only ONE mesh (one ~11us all-rank entry
# sync) is exposed after the PE phase ends
COLL0_N = 432
COLL1_N = 608

_F32 = mybir.dt.float32
_BF16 = mybir.dt.bfloat16
_FP8 = mybir.dt.float8e4

_CACHE: dict = {}


def _build():
    nc = bacc.Bacc(
        "TRN2", target_bir_lowering=False, debug=False, num_devices=N_CORES
    )
    embed8 = nc.dram_tensor("embed8", [B_LOC, D], _FP8, kind="ExternalInput").ap()
    yt = nc.dram_tensor("yt", [P, KT], _F32, kind="ExternalInput").ap()
    cent = nc.dram_tensor("cent", [CM, D], _F32, kind="ExternalInput").ap()
    out = nc.dram_tensor("out", [CM, D], _F32, kind="ExternalOutput").ap()

    with tile.TileContext(nc) as tc:
        with (
            tc.tile_pool(name="dram", bufs=1, space="DRAM") as dram,
            tc.tile_pool(name="const", bufs=1) as const_pool,
            tc.tile_pool(name="emb", bufs=KP) as emb_pool,
            tc.tile_pool(name="oh", bufs=KP) as oh_pool,
            tc.tile_pool(name="stage", bufs=8) as stage_pool,
            tc.tile_pool(name="psum", bufs=MT, space="PSUM") as psum_pool,
            tc.tile_pool(name="fin", bufs=3) as fin_pool,
        ):
            cc_ins = [
                dram.tile([C_PAD, COLL0_N], _BF16, name="cc_in0"),
                dram.tile([C_PAD, COLL1_N], _BF16, name="cc_in1"),
            ]
            cc_outs = [
                dram.tile([CM, COLL0_N], _BF16, name="cc_out0"),
                dram.tile([CM, COLL1_N], _BF16, name="cc_out1"),
            ]

            # iota row replicated down all 128 partitions: iota[p, c] = c
            iota = const_pool.tile([P, C_PAD], _F32)
            nc.gpsimd.iota(
                iota[:],
                pattern=[[1, C_PAD]],
                base=0,
                channel_multiplier=0,
                allow_small_or_imprecise_dtypes=True,
            )
            # all 32 k-tiles' labels in one DMA: y_all[:, k] = y[k*128:(k+1)*128]
            y_all = const_pool.tile([P, KT], _F32)
            nc.gpsimd.dma_start(out=y_all[:], in_=yt[:])

            emb_tiles = []
            oh_tiles = []
            for j in range(KP):
                emb_t = emb_pool.tile([P, 2, W], _FP8, name=f"emb{j}", tag="emb")
                oh_t = oh_pool.tile([P, 2, C_PAD], _FP8, name=f"oh{j}", tag="oh")
                for j2 in range(2):
                    k = 2 * j + j2
                    rows = slice(k * P, (k + 1) * P)
                    nc.vector.memset(emb_t[:, j2, 0:1], 1.0)  # count column
                    nc.vector.memset(emb_t[:, j2, 1 + D : W], 0.0)  # row pad
                    nc.sync.dma_start(
                        out=emb_t[:, j2, 1 : 1 + D], in_=embed8[rows, :]
                    )
                    nc.vector.tensor_scalar(
                        oh_t[:, j2, :],
                        iota[:],
                        y_all[:, k : k + 1],
                        None,
                        mybir.AluOpType.is_equal,
                    )
                emb_tiles.append(emb_t)
                oh_tiles.append(oh_t)

            # recip[:, 0:1] will hold 0.3 / count once chunk 0 has landed
            recip = fin_pool.tile([P, 1], _F32, name="recip", tag="recip", bufs=1)

            # pre-scale the centroid by 0.7 while the matmuls run, one tile
            # per collective chunk's dim range: chunk 0 = dims 0..COLL0_N-2,
            # chunk 1 = the rest (its cc cols carry no count column)
            cent07 = []
            for q, (d_lo, ncols) in enumerate(
                [(0, COLL0_N - 1), (COLL0_N - 1, D - (COLL0_N - 1))]
            ):
                c_sb = fin_pool.tile(
                    [P, ncols], _F32, name=f"c07_{q}", tag="c07", bufs=2
                )
                nc.gpsimd.dma_start(out=c_sb[:], in_=cent[:, d_lo : d_lo + ncols])
                nc.scalar.mul(c_sb[:], c_sb[:], 1.0 - FACTOR)
                cent07.append((c_sb, d_lo, ncols))

            def finalize(q, red_n, r_lo):
                # mean = sums * (0.3/count); out = mean + 0.7*centroid
                red = fin_pool.tile([P, red_n], _BF16, name=f"red{q}", tag="red")
                nc.sync.dma_start(out=red[:], in_=cc_outs[q][:])
                c_sb, d_lo, ncols = cent07[q]
                if q == 0:
                    cnt_f = fin_pool.tile([P, 1], _F32, name="cnt_f", bufs=1)
                    nc.vector.tensor_copy(out=cnt_f[:], in_=red[:, 0:1])
                    nc.vector.reciprocal(recip[:], cnt_f[:])
                    nc.vector.tensor_scalar(
                        recip[:], recip[:], FACTOR, None, mybir.AluOpType.mult
                    )
                t1 = fin_pool.tile([P, ncols], _F32, name=f"t1_{q}", tag="t1")
                nc.vector.tensor_scalar(
                    t1[:],
                    red[:, r_lo : r_lo + ncols],
                    recip[:, 0:1],
                    None,
                    mybir.AluOpType.mult,
                )
                out_sb = fin_pool.tile([P, ncols], _F32, name=f"o{q}", tag="o")
                nc.vector.tensor_tensor(
                    out=out_sb[:], in0=t1[:], in1=c_sb[:], op=mybir.AluOpType.add
                )
                nc.sync.dma_start(out=out[:, d_lo : d_lo + ncols], in_=out_sb[:])

            for p, (off, n) in enumerate(CHUNKS):
                psums = [
                    psum_pool.tile([P, n], _F32, name=f"ps{p}_{m}", tag="ps")
                    for m in range(MT)
                ]

                def mm(j, m, p=p, n=n, off=off, psums=psums):
                    nc.tensor.matmul(
                        psums[m][:],
                        lhsT=oh_tiles[j][:, :, m * P : (m + 1) * P],
                        rhs=emb_tiles[j][:, :, off : off + n],
                        start=(j == 0),
                        stop=(j == KP - 1),
                        perf_mode=mybir.MatmulPerfMode.DoubleRow,
                    )

                if p == 0:
                    # j-major: tracks the tile-generation pipeline (each
                    # matmul only needs pair j, not all 16)
                    for j in range(KP):
                        for m in range(MT):
                            mm(j, m)
                else:
                    # m-major: lower per-matmul overhead, staggered PSUM
                    # eviction so copies/DMAs overlap the next group
                    for m in range(MT):
                        for j in range(KP):
                            mm(j, m)

                # evict this pass into its collective buffer: pass 0 ->
                # cc_in0, passes 1+2 -> adjacent column ranges of cc_in1
                q = 0 if p == 0 else 1
                c_off = 0 if p == 0 else off - COLL0_N
                for m in range(MT):
                    sums_sb = stage_pool.tile(
                        [P, n], _BF16, name=f"sb{p}_{m}", tag="sums_sb"
                    )
                    # split pass-0 eviction across ACT and DVE so the first
                    # ReduceScatter triggers sooner
                    if p == 0 and m % 2 == 0:
                        nc.vector.tensor_copy(out=sums_sb[:], in_=psums[m][:])
                    else:
                        nc.scalar.copy(out=sums_sb[:], in_=psums[m][:])
                    # alternate DMA queues so the burst of 8 evictions at a
                    # pass boundary isn't serialized on one queue's issue rate
                    dma_eng = nc.sync if m % 2 == 0 else nc.gpsimd
                    dma_eng.dma_start(
                        out=cc_ins[q][m * P : (m + 1) * P, c_off : c_off + n],
                        in_=sums_sb[:],
                    )

                if p != 1:
                    nc.gpsimd.collective_compute(
                        "ReduceScatter",
                        mybir.AluOpType.add,
                        replica_groups=[list(range(N_CORES))],
                        ins=[cc_ins[q].opt()],
                        outs=[cc_outs[q].opt()],
                    )
                    finalize(q, COLL0_N if q == 0 else COLL1_N, 1 if q == 0 else 0)

    nc.compile()
    return nc


def get_nc():
    if "nc" not in _CACHE:
        _CACHE["nc"] = _build()
    return _CACHE["nc"]


def make_in_maps(embed: np.ndarray, y: np.ndarray, centroid: np.ndarray):
    fp8_np = mybir.dt.np(_FP8)
    embed8 = np.ascontiguousarray(embed, dtype=np.float32).astype(fp8_np)
    y_f = np.asarray(y).astype(np.float32)
    cent_pad = np.zeros((C_PAD, D), dtype=np.float32)
    cent_pad[:C] = np.asarray(centroid, dtype=np.float32)
    in_maps = []
    for i in range(N_CORES):
        y_loc = y_f[i * B_LOC : (i + 1) * B_LOC]
        in_maps.append(
            {
                "embed8": embed8[i * B_LOC : (i + 1) * B_LOC],
                # yt[:, k] = y_loc[k*128:(k+1)*128]
                "yt": np.ascontiguousarray(y_loc.reshape(KT, P).T),
                "cent": np.ascontiguousarray(cent_pad[i * CM : (i + 1) * CM]),
            }
        )
    return in_maps


def kernel(embed: np.ndarray, y: np.ndarray, centroid: np.ndarray) -> np.ndarray:
    nc = get_nc()
    in_maps = make_in_maps(embed, y, centroid)
    res = run_bass_kernel_spmd(nc, in_maps, core_ids=list(range(N_CORES)))
    full = np.concatenate([res.results[i]["out"] for i in range(N_CORES)], axis=0)
    return np.ascontiguousarray(full[:C]).astype(np.float32)



# revision 4
# speedup vs baseline: 2.7934x; 2.7934x over previous
"""Trainium2 Bass kernel for the Centroid (segment_reduce) problem.

new_centroid = 0.3 * (segment_sum(embed, y) / counts) + 0.7 * centroid
  embed [32768, 1024] f32, y [32768] int64 (0..999), centroid [1000, 1024] f32

Strategy (8 NeuronCores, CLASS-sharded via host-side sort — no collective):
  - host sorts the batch by label and gives core i ALL rows whose label is
    in [125*i, 125*(i+1)) (125 classes per core, 8*125 = 1000). Row counts
    per core are ~4096 +- 64 (multinomial), padded to a common multiple of
    256 with zero rows (count flag 0), so one SPMD program serves all cores.
  - each core's one-hot spans only 125 classes -> a single 128-class M-tile:
    sums[c, d] = sum_b onehot[b, c] * embed[b, d] runs as KP DoubleRow fp8
    matmuls accumulating in ONE PSUM region [128, 1025] (cols 0..1023 =
    dims, col 1024 = count; the count column of the rhs is 1.0 for real
    rows / 0.0 for pad rows, so counts come free with the sums).
  - labels are passed relative to the core's class base, so the iota /
    is_equal one-hot build is identical on every core (SPMD-clean).
  - cores own disjoint classes => NO cross-core reduction of any kind.
    finalize per core: out = (sums * (0.3/count)) + 0.7*centroid for its
    125 classes; host concatenates the 8 [125, 1024] shards. Zero
    collectives, zero bf16 rounding of partial sums.
"""

import numpy as np

import concourse.bacc as bacc
import concourse.mybir as mybir
import concourse.tile as tile
from concourse.bass_utils import run_bass_kernel_spmd

N_CORES = 8
C = 1000  # real classes
CPC = C // N_CORES  # 125 classes owned per core
D = 1024  # embed dim
B = 32768  # total batch
P = 128
W_IN = D + 1  # 1024 dims + count column (col 1024)
W_SB = 1040  # SBUF row stride, mult of 16 (DoubleRow step constraint)
FACTOR = 0.3

_F32 = mybir.dt.float32
_FP8 = mybir.dt.float8e4

_CACHE: dict = {}


def _build(kp: int):
    """kp = number of 256-row DoubleRow pairs per core."""
    kt = 2 * kp
    nc = bacc.Bacc(
        "TRN2", target_bir_lowering=False, debug=False, num_devices=N_CORES
    )
    emb8 = nc.dram_tensor("emb8", [kt * P, W_IN], _FP8, kind="ExternalInput").ap()
    yt = nc.dram_tensor("yt", [P, kt], _F32, kind="ExternalInput").ap()
    iota_in = nc.dram_tensor("iota", [P, P], _F32, kind="ExternalInput").ap()
    cent = nc.dram_tensor("cent", [CPC, D], _F32, kind="ExternalInput").ap()
    out = nc.dram_tensor("out", [CPC, D], _F32, kind="ExternalOutput").ap()

    with tile.TileContext(nc) as tc:
        with (
            tc.tile_pool(name="const", bufs=1) as const_pool,
            tc.tile_pool(name="emb", bufs=kp) as emb_pool,
            tc.tile_pool(name="oh", bufs=kp) as oh_pool,
            tc.tile_pool(name="psum", bufs=1, space="PSUM") as psum_pool,
            tc.tile_pool(name="fin", bufs=8) as fin_pool,
        ):
            # iota[p, c] = c (DMA'd constant: gpsimd.iota costs ~1.8us)
            iota = const_pool.tile([P, P], _F32)
            nc.gpsimd.dma_start(out=iota[:], in_=iota_in[:])
            # window-relative labels: y_all[p, k] = y_rel[k*128 + p]
            y_all = const_pool.tile([P, kt], _F32)
            nc.gpsimd.dma_start(out=y_all[:], in_=yt[:])

            # 0.7 * centroid, prescaled while the matmuls run
            cent07 = []
            for q in range(2):
                c_sb = fin_pool.tile(
                    [P, 512], _F32, name=f"c07_{q}", tag="c07", bufs=2
                )
                nc.gpsimd.dma_start(
                    out=c_sb[0:CPC, :], in_=cent[:, q * 512 : (q + 1) * 512]
                )
                nc.vector.tensor_scalar(
                    c_sb[0:CPC, :],
                    c_sb[0:CPC, :],
                    1.0 - FACTOR,
                    None,
                    mybir.AluOpType.mult,
                )
                cent07.append(c_sb)

            emb_tiles = []
            oh_tiles = []
            for j in range(kp):
                emb_t = emb_pool.tile([P, 2, W_SB], _FP8, name=f"emb{j}", tag="emb")
                oh_t = oh_pool.tile([P, 2, P], _FP8, name=f"oh{j}", tag="oh")
                for j2 in range(2):
                    k = 2 * j + j2
                    rows = slice(k * P, (k + 1) * P)
                    # cols 1025..1039 of emb_t are never read by any matmul
                    dma_eng = nc.sync if k % 2 == 0 else nc.scalar
                    dma_eng.dma_start(
                        out=emb_t[:, j2, 0:W_IN], in_=emb8[rows, :]
                    )
                    nc.vector.tensor_scalar(
                        oh_t[:, j2, :],
                        iota[:],
                        y_all[:, k : k + 1],
                        None,
                        mybir.AluOpType.is_equal,
                    )
                emb_tiles.append(emb_t)
                oh_tiles.append(oh_t)

            ps0 = psum_pool.tile([P, 512], _F32, name="ps0", tag="ps0")
            ps1 = psum_pool.tile([P, 512], _F32, name="ps1", tag="ps1")
            ps2 = psum_pool.tile([P, 1], _F32, name="ps2", tag="ps2")
            chunks = [(ps0, 0, 512), (ps1, 512, 512), (ps2, 1024, 1)]

            for j in range(kp):
                for ps, off, n in chunks:
                    nc.tensor.matmul(
                        ps[:],
                        lhsT=oh_tiles[j][:, :, :],
                        rhs=emb_tiles[j][:, :, off : off + n],
                        start=(j == 0),
                        stop=(j == kp - 1),
                        perf_mode=mybir.MatmulPerfMode.DoubleRow,
                    )

            # recip = 0.3 / count
            recip = fin_pool.tile([P, 1], _F32, name="recip", bufs=1)
            nc.vector.reciprocal(recip[:], ps2[:])
            nc.vector.tensor_scalar(
                recip[:], recip[:], FACTOR, None, mybir.AluOpType.mult
            )
            for q, (ps, off, n) in enumerate(chunks[:2]):
                out_sb = fin_pool.tile([P, 512], _F32, name=f"o{q}", tag="o")
                # out = (sums * (0.3/count)) + 0.7*centroid, single fused op
                nc.vector.scalar_tensor_tensor(
                    out_sb[0:CPC, :],
                    ps[0:CPC, :],
                    recip[0:CPC, 0:1],
                    cent07[q][0:CPC, :],
                    mybir.AluOpType.mult,
                    mybir.AluOpType.add,
                )
                dma_eng = nc.sync if q == 0 else nc.scalar
                dma_eng.dma_start(
                    out=out[:, off : off + n], in_=out_sb[0:CPC, :]
                )

    nc.compile()
    return nc


def get_nc(kp: int):
    if kp not in _CACHE:
        _CACHE[kp] = _build(kp)
    return _CACHE[kp]


def prepare(embed: np.ndarray, y: np.ndarray, centroid: np.ndarray):
    """Sort batch by label, shard class-aligned, pad, quantize. Returns
    (nc, in_maps)."""
    y = np.asarray(y).astype(np.int64).ravel()
    order = np.argsort(y, kind="stable")
    ys = y[order]
    bounds = np.searchsorted(ys, np.arange(0, C + 1, CPC))
    n_max = int(np.diff(bounds).max())
    kp = max((n_max + 255) // 256, 1)
    rows = kp * 256
    kt = 2 * kp

    fp8 = mybir.dt.np(_FP8)
    embf = np.asarray(embed, dtype=np.float32)
    centf = np.asarray(centroid, dtype=np.float32)
    one8 = np.float32(1.0).astype(fp8)

    in_maps = []
    iota_np = np.ascontiguousarray(
        np.tile(np.arange(P, dtype=np.float32), (P, 1))
    )
    for i in range(N_CORES):
        lo, hi = int(bounds[i]), int(bounds[i + 1])
        n = hi - lo
        idx = order[lo:hi]
        emb8 = np.zeros((rows, W_IN), dtype=fp8)
        emb8[:n, :D] = embf[idx].astype(fp8)
        emb8[:n, D] = one8
        y_rel = np.zeros(rows, dtype=np.float32)
        y_rel[:n] = (ys[lo:hi] - CPC * i).astype(np.float32)
        in_maps.append(
            {
                "emb8": emb8,
                "yt": np.ascontiguousarray(y_rel.reshape(kt, P).T),
                "iota": iota_np,
                "cent": np.ascontiguousarray(centf[CPC * i : CPC * (i + 1)]),
            }
        )
    return get_nc(kp), in_maps


def assemble(res) -> np.ndarray:
    full = np.concatenate(
        [res.results[i]["out"] for i in range(N_CORES)], axis=0
    )
    return np.ascontiguousarray(full).astype(np.float32)


def kernel(embed: np.ndarray, y: np.ndarray, centroid: np.ndarray) -> np.ndarray:
    nc, in_maps = prepare(embed, y, centroid)
    res = run_bass_kernel_spmd(nc, in_maps, core_ids=list(range(N_CORES)))
    return assemble(res)
